# revision 72
# baseline (speedup 1.0000x reference)
"""Binary KL divergence sum on 8 Trainium2 NeuronCores.

Reference math (per element, summed over all 2**25 elements):
    kl = p*(ln p - ln q) + (1-p)*(ln(1-p) - ln(1-q))

Rewritten with t1 = ln p - ln q, t2 = ln(1-p) - ln(1-q):
    sum(kl) = sum(t2) + sum(p * (t1 - t2))

Sharding: element axis split evenly across 8 cores; each core reduces its
partials into PSUM via PE matmuls against a ones vector; host sums 8*512
partials.

Per-core pipeline, processed in "mega tiles" of [128, 2*M] fp32 laid out
as [p-block | q-block] (sub-chunk DMAs fill each block):
  DMA  : p subchunks -> PQ[:, :M], q subchunks -> PQ[:, M:]
  ACT  : L  = Ln(PQ)        -> fp16 ([ln p | ln q], one instr per mega)
  ACT  : L1 = Ln(1 - PQ)    -> fp16
  DVE  : p16 = cast(PQ[:, :M])            (fp32->fp16, 2x)
  DVE  : t1 = L[:, :M] - L[:, M:]         (fp16 TT, 2x)
  DVE  : t2 = L1[:, :M] - L1[:, M:]       (fp16 TT, 2x)
  DVE  : d  = t1 - t2
  DVE  : m  = p16 * d
  PE   : acc += ones^T @ t2 ; acc += ones^T @ m   (PSUM fp32)

The scalar engine is the bottleneck (4 Ln evaluations per element pair at
1 elem/lane/cycle); mega tiles amortize ACT instruction overhead and deep
DMA buffering keeps it fed. Small first/last megas shorten pipeline
fill/drain.
"""

import numpy as np

import concourse.bass as bass
import concourse.bacc as bacc
import concourse.mybir as mybir
from concourse import bass_utils
from concourse.tile import TileContext

N = 33554432
NCORES = 8
PER = N // NCORES   # 4194304 elements per core per tensor
P = 128
L = PER // P        # 32768 free elements per partition per tensor

AF = mybir.ActivationFunctionType
OP = mybir.AluOpType
DT = mybir.dt

_NC_CACHE = {}

NRED = 512  # one PSUM bank of fp32: matmul free-dim chunk

# Mega-tile schedule: per-tensor widths, each mega split into DMA subchunks.
# Small megas first (ACT starts after ~0.25 MB of DMA) and last (short drain).
# One DMA per mega per tensor: dma_start issue costs ~0.6us of sequencer
# time each, and fat partition rows (up to 12KB) move faster.
MEGAS = (
    [[512], [1536], [1536]]
    + [[3072]] * 8
    + [[2048], [1536], [512], [512]]
)
assert sum(sum(m) for m in MEGAS) == L
SMALL = 1536   # megas up to this width use the small landing pool




def _build_nc():
    nc = bacc.Bacc("TRN2", target_bir_lowering=False, debug=False,
                   num_devices=NCORES)
    inp = nc.dram_tensor("input", [PER], DT.float32, kind="ExternalInput")
    tgt = nc.dram_tensor("target", [PER], DT.float32, kind="ExternalInput")
    out = nc.dram_tensor("partials", [NRED], DT.float32,
                         kind="ExternalOutput")

    p_flat = inp.ap()
    q_flat = tgt.ap()
    out_view = out.ap().rearrange("(o n) -> o n", o=1)

    def mm_chunks(w):
        out = []
        while w > 0:
            c = min(w, NRED)
            out.append(c)
            w -= c
        return out

    n_mm = 2 * sum(len(mm_chunks(sum(m))) for m in MEGAS)

    with TileContext(nc) as tc:
        with (
            tc.tile_pool(name="io32", bufs=3) as io32,
            tc.tile_pool(name="ios", bufs=3) as ios,
            tc.tile_pool(name="l16", bufs=5) as l16,
            tc.tile_pool(name="s16", bufs=2) as s16,
            tc.tile_pool(name="cst", bufs=1) as cst,
            tc.tile_pool(name="ps", bufs=1, space="PSUM") as psp,
        ):
            ones = cst.tile([P, 1], DT.float16, tag="ones")
            nc.vector.memset(ones[:], 1.0)
            acc = psp.tile([1, NRED], DT.float32, tag="acc")
            osb = cst.tile([1, NRED], DT.float32, tag="osb")
            # zero-init the accumulator; all matmuls accumulate (start=False)
            # so variable-width chunks can land on any column range.
            nc.vector.memset(acc[:], 0.0)

            # Dummy 1-element Ln so the ACT table load happens while the
            # first DMAs are still in flight (osb is overwritten later).
            warm = cst.tile([1, 1], DT.float32, tag="warm")
            nc.vector.memset(warm[:], 0.5)
            nc.scalar.activation(osb[0:1, 0:1], warm[:], AF.Ln)

            mm = 0

            def mm_accum(src, w):
                nonlocal mm
                off = 0
                for c in mm_chunks(w):
                    nc.tensor.matmul(
                        acc[:, 0:c], ones[:], src[:, off:off + c],
                        start=False, stop=(mm == n_mm - 1),
                        skip_group_check=True)
                    off += c
                    mm += 1

            base = 0
            for subs in MEGAS:
                M = sum(subs)
                pool = ios if M <= SMALL else io32
                pq = pool.tile([P, 2 * M], DT.float32, tag="pq")
                # p subchunks into [0:M], q subchunks into [M:2M]
                off = 0
                for w in subs:
                    nc.sync.dma_start(
                        pq[:, off:off + w],
                        p_flat[base + P * off:base + P * (off + w)]
                        .rearrange("(p f) -> p f", p=P))
                    nc.sync.dma_start(
                        pq[:, M + off:M + off + w],
                        q_flat[base + P * off:base + P * (off + w)]
                        .rearrange("(p f) -> p f", p=P))
                    off += w
                base += P * M

                Lt = l16.tile([P, 2 * M], DT.float16, tag="L")
                L1 = l16.tile([P, 2 * M], DT.float16, tag="L")
                nc.scalar.activation(Lt[:], pq[:], AF.Ln)
                nc.scalar.activation(L1[:], pq[:], AF.Ln, bias=1.0,
                                     scale=-1.0)

                p16 = s16.tile([P, M], DT.float16, tag="p16")
                nc.vector.tensor_copy(p16[:], pq[:, 0:M])

                t1 = s16.tile([P, M], DT.float16, tag="t1")
                nc.vector.tensor_tensor(t1[:], Lt[:, 0:M], Lt[:, M:2 * M],
                                        OP.subtract)

                t2 = s16.tile([P, M], DT.float16, tag="t2")
                nc.vector.tensor_tensor(t2[:], L1[:, 0:M], L1[:, M:2 * M],
                                        OP.subtract)

                # d and m reuse t1's storage (in-place elementwise)
                nc.vector.tensor_tensor(t1[:], t1[:], t2[:], OP.subtract)
                nc.vector.tensor_tensor(t1[:], p16[:], t1[:], OP.mult)

                mm_accum(t2, M)  # sum(t2)
                mm_accum(t1, M)  # sum(p*(t1-t2))

            # stage the accumulator to SBUF and write out
            nc.vector.tensor_copy(osb[:], acc[:])
            nc.sync.dma_start(out_view[:], osb[:])

    nc.compile()
    return nc


def _get_nc():
    if "nc" not in _NC_CACHE:
        _NC_CACHE["nc"] = _build_nc()
    return _NC_CACHE["nc"]


def kernel(input, target, _trace=False):
    input = np.ascontiguousarray(np.asarray(input), dtype=np.float32)
    target = np.ascontiguousarray(np.asarray(target), dtype=np.float32)
    nc = _get_nc()
    in_maps = [
        {
            "input": input[c * PER:(c + 1) * PER],
            "target": target[c * PER:(c + 1) * PER],
        }
        for c in range(NCORES)
    ]
    res = bass_utils.run_bass_kernel_spmd(
        nc, in_maps, core_ids=list(range(NCORES)), trace=_trace)
    total = np.float64(0.0)
    for c in range(NCORES):
        total += res.results[c]["partials"].astype(np.float64).sum()
    out = np.asarray(total, dtype=np.float32)
    if _trace:
        return out, res
    return out
